# revision 13
# baseline (speedup 1.0000x reference)
"""Trainium2 Bass kernel for nn_CausalSelfAttention_74268574482879.

The reference module's attention scores are overwritten by the causal mask
(q/k are discarded), so softmax weights are uniform over positions <= t:
    y = cummean_T(x) @ W,   W = w_attn[:, 1024:1536] @ w_proj  (host-folded)

Distribution: the 4096 rows of (B*T) are split into 8 chunks of 512 rows,
one per NeuronCore.  The only cross-chunk dependency is the column-sum of
all preceding rows in the same batch element; the host passes that tiny
(512,) halo vector per core while slicing the shards.

Per-core dataflow (~40 instructions).  The binding constraint on TRN2
with all 8 cores streaming is the per-core aggregate DMA rate
(~125 GB/s across the three trigger queues: sync/SP, scalar/Activation,
gpsimd), so the kernel is organized as a need-ordered input stream with
all compute hidden behind it:
  - x^T arrives pre-transposed/packed bf16 (features on partitions), one
    DMA per 128-feature block, interleaved with per-block W DMAs across
    the three queues in exactly the order the pipeline consumes them;
    the last arrival is a W block (shortest tail: one PE round, no scan)
  - 4 tensor_tensor_scan ops on the DVE compute the running column-sum
    along time (fp32 state), seeded with the halo via `initial`
  - 16 bf16 matmuls accumulate psY_j in 4 PSUM banks; rounds are ordered
    by W-block arrival, stop flag on the last-arriving block
  - eviction fuses the deferred 1/(t+1) row scale (per-partition scalar),
    alternating Activation/DVE; y leaves as two packed bf16 row-pair
    DMAs that the host unpacks/upcasts
"""

import numpy as np
from ml_dtypes import bfloat16

import concourse.bass as bass
import concourse.bacc as bacc
import concourse.mybir as mybir
import concourse.tile as tile
from concourse import bass_utils

N_CORES = 8
B, T, C = 2, 2048, 512
CHUNK = 512               # rows of flattened (B*T) per core
P = 128
NT = CHUNK // P           # 4 row-tiles per chunk
NI = C // P               # 4 col-tiles of the 512 feature dim
H = CHUNK // 2            # half-scan length
F32 = mybir.dt.float32
BF16 = mybir.dt.bfloat16

MODE = ["bf16o"]          # "bf16o" (bf16 out) | "bf16" (f32 out)
TRACE = [False]
LAST_RESULT = [None]
_STATE = {}


def _build_nc(mode):
    out_bf16 = mode == "bf16o"
    y_dt = BF16 if out_bf16 else F32

    nc = bacc.Bacc(
        "TRN2", target_bir_lowering=False, debug=False, num_devices=N_CORES
    )

    xt_d = nc.dram_tensor("xt", (P, NI, CHUNK), BF16, kind="ExternalInput")
    w_d = nc.dram_tensor("w", (P, NI, C), BF16, kind="ExternalInput")
    sc_d = nc.dram_tensor("sc", (P, NI + NT), F32, kind="ExternalInput")
    # y is packed partition-major ([p, j, c] = row j*P+p); host unpacks
    y_d = nc.dram_tensor("y", (P, NT, C), y_dt, kind="ExternalOutput")

    xt_ap, w_ap, sc_ap, y_ap = xt_d.ap(), w_d.ap(), sc_d.ap(), y_d.ap()
    ADD = mybir.AluOpType.add
    BYP = mybir.AluOpType.bypass

    with tile.TileContext(nc) as tc:
        with (
            tc.tile_pool(name="io", bufs=1) as io,
            tc.tile_pool(name="ps", bufs=1, space="PSUM") as psp,
        ):
            # ---- inputs; xt blocks spread over three queues so scans
            # start as early as the per-queue packet rate allows ----
            # need-ordered streaming: each queue delivers tensors in the
            # order the pipeline consumes them (xt_i before W_i; big W
            # split per block so round 0 is not gated on all of W)
            sc_sb = io.tile([P, NI + NT], F32, name="sc_sb")
            xt_sb = io.tile([P, NI, CHUNK], BF16, name="xt_sb")
            w_sb = io.tile([P, NI, C], BF16, name="w_sb")
            nc.scalar.dma_start(sc_sb[:], sc_ap[:, :])
            nc.sync.dma_start(xt_sb[:, 0, :], xt_ap[:, 0, :])
            nc.gpsimd.dma_start(w_sb[:, 0, :], w_ap[:, 0, :])
            nc.scalar.dma_start(xt_sb[:, 1, :], xt_ap[:, 1, :])
            nc.sync.dma_start(xt_sb[:, 3, :], xt_ap[:, 3, :])
            nc.gpsimd.dma_start(xt_sb[:, 2, :], xt_ap[:, 2, :])
            nc.scalar.dma_start(w_sb[:, 1, :], w_ap[:, 1, :])
            nc.sync.dma_start(w_sb[:, 3, :], w_ap[:, 3, :])
            nc.gpsimd.dma_start(w_sb[:, 2, :], w_ap[:, 2, :])

            # ---- stage A: A^T[ci, t] = halo_ci + cumsum_t x^T[ci, t] ----
            A_sb = io.tile([P, NI, CHUNK], BF16, name="A_sb")
            for i in (0, 1, 3, 2):   # xt arrival order across the queues
                nc.vector.tensor_tensor_scan(
                    A_sb[:, i, :],
                    xt_sb[:, i, :],
                    xt_sb[:, i, :],
                    sc_sb[:, i : i + 1],
                    ADD,
                    BYP,
                )

            # ---- stage Y: psY_j = sum_i A^T[ci, tj]^T @ W[ci, :] ----
            psy = [
                psp.tile([P, C], F32, name=f"psy{j}", tag=f"psy{j}")
                for j in range(NT)
            ]
            # rounds ordered by W-block arrival; the last round (i=2) has
            # the shortest post-arrival chain (no scan behind it)
            I_ORDER = (0, 1, 3)
            I_LAST = 2

            def mm(j, i):
                nc.tensor.matmul(
                    psy[j][:],
                    A_sb[:, i, j * P : (j + 1) * P],
                    w_sb[:, i, :],
                    start=(i == 0),
                    stop=(i == I_LAST),
                )

            y_sb = io.tile([P, NT, C], y_dt, name="y_sb")

            def evict(j):
                scol = sc_sb[:, NI + j : NI + j + 1]
                if j % 2 == 0:
                    nc.scalar.mul(y_sb[:, j, :], psy[j][:], scol)
                else:
                    nc.vector.tensor_scalar_mul(y_sb[:, j, :], psy[j][:], scol)

            for i in I_ORDER:
                mm(0, i)
                mm(1, i)
                mm(2, i)
                mm(3, i)
            for j in range(NT):
                mm(j, I_LAST)
                evict(j)
                if j == 1:
                    nc.sync.dma_start(y_ap[:, 0:2, :], y_sb[:, 0:2, :])
                elif j == 3:
                    nc.gpsimd.dma_start(y_ap[:, 2:4, :], y_sb[:, 2:4, :])

    nc.compile()
    return nc


def _get_nc():
    key = MODE[0]
    if key not in _STATE:
        _STATE[key] = _build_nc(key)
    return _STATE[key]


def _prepare_in_maps(x, w_attn, w_proj):
    x = np.asarray(x, dtype=np.float32)
    w_attn = np.asarray(w_attn, dtype=np.float32)
    w_proj = np.asarray(w_proj, dtype=np.float32)
    w = (w_attn[:, 2 * C : 3 * C] @ w_proj).astype(np.float32)
    wpk = np.ascontiguousarray(
        w.reshape(NI, P, C).transpose(1, 0, 2)
    ).astype(bfloat16)

    in_maps = []
    for core in range(N_CORES):
        b, tc = divmod(core, T // CHUNK)
        goff = tc * CHUNK
        chunk = x[b, goff : goff + CHUNK, :]
        # (P, NI, CHUNK): features on partitions, time on the free axis
        xt = np.ascontiguousarray(
            chunk.T.reshape(NI, P, CHUNK).transpose(1, 0, 2)
        ).astype(bfloat16)
        # halo: column-sum of all earlier rows in this batch element
        p = x[b, :goff, :].sum(axis=0, dtype=np.float32) if goff else np.zeros(
            C, np.float32
        )
        # scv[r, tt] = 1/(global_row+1) for row tt*P + r of this chunk
        scale = (1.0 / (goff + np.arange(1, CHUNK + 1))).astype(np.float32)
        sc = np.concatenate(
            [p.reshape(NI, P).T, scale.reshape(NT, P).T], axis=1
        ).astype(np.float32)
        in_maps.append({"xt": xt, "w": wpk, "sc": sc})
    return in_maps


def kernel(x, w_attn, w_proj):
    nc = _get_nc()
    in_maps = _prepare_in_maps(x, w_attn, w_proj)
    res = bass_utils.run_bass_kernel_spmd(
        nc, in_maps, core_ids=list(range(N_CORES)), trace=TRACE[0]
    )
    LAST_RESULT[0] = res
    y = np.empty((B, T, C), np.float32)
    for core in range(N_CORES):
        b, tc = divmod(core, T // CHUNK)
        yp = np.asarray(res.results[core]["y"], dtype=np.float32)
        y[b, tc * CHUNK : (tc + 1) * CHUNK, :] = yp.transpose(1, 0, 2).reshape(
            CHUNK, C
        )
    return y
